# revision 101
# baseline (speedup 1.0000x reference)
"""Trainium2 Bass kernel for LSTM-actor network (T=64, B=2048, OBS=48, H=256).

Strategy: data-parallel over batch B across 8 NeuronCores (256 envs/core).
V2: fp8 DoubleRow recurrent/input gate matmuls (pair layout == DR layout),
bf16 elementwise state so DVE ops hit the 2x/4x perf modes, merged ACT ops
(tanh(f|i) in one pass, g-gate weights predoubled so tanh(g|..) shares the
0.5 scale), biases for heads via per-partition scalar APs, bf16 broadcast
rows for keep/rk. MLP (LN -> 512 -> 256 -> heads) in bf16, pipelined LAG
steps behind the scan. Output feature-major [14, T*256] per core.
"""
import sys, os
sys.path.insert(0, "/opt/trn_rl_repo")
import numpy as np
import ml_dtypes
from contextlib import ExitStack

import concourse.bass as bass
import concourse.bacc as bacc
import concourse.tile as tile
from concourse import mybir
from concourse.bass_utils import run_bass_kernel_spmd

F32 = mybir.dt.float32
BF16 = mybir.dt.bfloat16
F8 = mybir.dt.float8e4
I32 = mybir.dt.int32
F32R = mybir.dt.float32r
AF = mybir.ActivationFunctionType
OP = mybir.AluOpType
PM = mybir.MatmulPerfMode

T, B, OBS, H, M1, M2, A = 64, 2048, 48, 256, 512, 256, 12
NC_N = 8
BL = B // NC_N          # 256 envs per core
G4 = 4 * H              # 1024
LOG2PI = float(np.log(2.0 * np.pi))
LN_EPS = 1e-5
BIG = 30.0
C_LOGP = -(A / 2.0) * LOG2PI          # logp = -s + C_LOGP
C_ENT = A * (0.5 + 0.5 * LOG2PI)      # ent  =  s + C_ENT

SC = 4.0                # fp8 operand prescale (both sides -> psum x16)
GSC = 0.5 / 16.0        # tanh input scale for f,i,o (g-block predoubled)

RING = 14
LAG = 12                # MLP stage-1 lag
ZCH = 16                # z0 staging chunk (steps)
PYCHUNK = [2, 3, 0, 1, 4, 5, 6, 7]   # psum block -> pytorch 128-row chunk


def _bcast_ap(src):
    """DRAM row [1, n] -> partition-broadcast AP [[0,128],[...]]"""
    return bass.AP(tensor=src.tensor, offset=src.offset, ap=[[0, 128]] + src.ap[1:])


def _pair3(ap_2d):
    return ap_2d.rearrange("p (k b) -> p k b", k=2)


def _row3(ap_2d):
    """[128, 256] -> [128, 2(bcast), 256] 0-stride view"""
    return bass.AP(tensor=ap_2d.tensor, offset=ap_2d.offset,
                   ap=[ap_2d.ap[0], [0, 2], ap_2d.ap[1]])


def build_nc():
    nc = bacc.Bacc(None, target_bir_lowering=False)
    dt = nc.dram_tensor
    # per-core inputs
    z0_d = dt("z0", [32, T * 512], F8, kind="ExternalInput")      # [32, t, 2, 256]
    mbrow_d = dt("mbrow", [T, BL], BF16, kind="ExternalInput")    # 2*keep[t+1]
    hm0_d = dt("hm0", [128, 512], F8, kind="ExternalInput")       # SC*h0*keep0 pair
    c0_d = dt("c0p", [128, 512], BF16, kind="ExternalInput")      # 2*c0 pair
    # replicated weights
    W0_d = dt("W0dr", [32, 2048], F8, kind="ExternalInput")       # [32, 2, 1024]
    Wh_d = dt("Whdr", [128, 2048], F8, kind="ExternalInput")      # [128, 2, 1024]
    W1_d = dt("W1", [H, M1], BF16, kind="ExternalInput")
    W2_d = dt("W2", [M1, M2], BF16, kind="ExternalInput")
    Whd_d = dt("Whd", [M2, 128], BF16, kind="ExternalInput")
    b1_d = dt("b1r", [1, M1], F32R, kind="ExternalInput")
    b2_d = dt("b2r", [1, M2], F32R, kind="ExternalInput")
    bhd_d = dt("bhdr", [1, 128], F32R, kind="ExternalInput")
    onesrow_d = dt("onesrow", [1, BL], F32R, kind="ExternalInput")
    onesmat_d = dt("onesmat", [128, 128], BF16, kind="ExternalInput")
    # internal scratch
    stats_dram = dt("stats_scr", [T, 512], F32, kind="Internal")
    rk_dram = dt("rk_scr", [T, 512], BF16, kind="Internal")
    s_dram = dt("s_scr", [T, BL], F32, kind="Internal")
    # output (feature-major)
    out_d = dt("out", [14, T * BL], F32, kind="ExternalOutput")

    with ExitStack() as ctx:
        tc = ctx.enter_context(tile.TileContext(nc))
        singles = ctx.enter_context(tc.tile_pool(name="singles", bufs=1))
        zpool = ctx.enter_context(tc.tile_pool(name="zpool", bufs=2))
        spool = ctx.enter_context(tc.tile_pool(name="spool", bufs=2))
        mpool = ctx.enter_context(tc.tile_pool(name="mpool", bufs=3))
        stpool = ctx.enter_context(tc.tile_pool(name="stpool", bufs=2))
        gps = ctx.enter_context(tc.tile_pool(name="gps", bufs=1, space="PSUM"))
        yps = ctx.enter_context(tc.tile_pool(name="yps", bufs=1, space="PSUM"))
        hdps = ctx.enter_context(tc.tile_pool(name="hdps", bufs=1, space="PSUM"))

        # ---- load inputs needed at t=0 first, then weights ----
        zc_cur = zpool.tile([32, ZCH * 512], F8, tag="zc")
        nc.sync.dma_start(out=zc_cur, in_=z0_d[:, 0:ZCH * 512])
        # state: ctile[:, 0:512] = c2 (=2c), ctile[:, 512:1024] = tanh(g) slot
        ctile = singles.tile([128, 1024], BF16)
        nc.gpsimd.dma_start(out=ctile[:, 0:512], in_=c0_d[:, :])
        hm8_init = singles.tile([128, 2, 256], F8)
        nc.gpsimd.dma_start(out=hm8_init, in_=hm0_d[:, :].rearrange("p (k b) -> p k b", k=2))
        mb_first = spool.tile([128, BL], BF16, tag="mb")
        nc.gpsimd.dma_start(out=mb_first, in_=_bcast_ap(mbrow_d[0:1, :]))
        W0s = singles.tile([32, 2, 1024], F8)
        nc.sync.dma_start(out=W0s, in_=W0_d[:, :].rearrange("p (k f) -> p k f", k=2))
        Whs = singles.tile([128, 2, 1024], F8)
        nc.gpsimd.dma_start(out=Whs, in_=Wh_d[:, :].rearrange("p (k f) -> p k f", k=2))
        onesmat = singles.tile([128, 128], BF16)
        nc.sync.dma_start(out=onesmat, in_=onesmat_d[:, :])
        # MLP weights (first needed at t = LAG)
        W1s = [singles.tile([128, M1], BF16, name=f"W1s{_k}") for _k in range(2)]
        for k in range(2):
            nc.sync.dma_start(out=W1s[k], in_=W1_d[k * 128:(k + 1) * 128, :])
        W2s = [singles.tile([128, M2], BF16, name=f"W2s{_k}") for _k in range(4)]
        for k in range(4):
            nc.gpsimd.dma_start(out=W2s[k], in_=W2_d[k * 128:(k + 1) * 128, :])
        Whds = [singles.tile([128, 128], BF16, name=f"Whds{_k}") for _k in range(2)]
        for k in range(2):
            nc.sync.dma_start(out=Whds[k], in_=Whd_d[k * 128:(k + 1) * 128, :])
        b1s = singles.tile([1, M1], F32R)
        nc.gpsimd.dma_start(out=b1s, in_=b1_d[:, :])
        b2s = singles.tile([1, M2], F32R)
        nc.sync.dma_start(out=b2s, in_=b2_d[:, :])
        bhds = singles.tile([1, 128], F32R)
        nc.gpsimd.dma_start(out=bhds, in_=bhd_d[:, :])
        onesrow = singles.tile([1, BL], F32R)
        nc.sync.dma_start(out=onesrow, in_=onesrow_d[:, :])
        ones12 = singles.tile([12, 1], BF16)
        nc.vector.memset(ones12, 1.0)
        h_ring = [singles.tile([128, 512], BF16, name=f"hring{_k}") for _k in range(RING)]

        state = dict(hm8=hm8_init, mb=mb_first, zc=zc_cur, zc_next=None)
        hsq_tiles = [singles.tile([128, 512], BF16, name=f"hsqt{_k}") for _k in range(2)]
        rkb_tiles = {}
        z_tiles = {}
        e1_tiles = {}
        e2_tiles = {}

        def gates(t):
            gA = gps.tile([128, 1024], F32, tag="gA")
            gB = gps.tile([128, 1024], F32, tag="gB")
            zoff = (t % ZCH) * 512
            zt = state["zc"][:, zoff:zoff + 512].rearrange("p (k b) -> p k b", k=2)
            hm8 = state["hm8"]
            for blk in range(8):
                gt = gA if blk < 4 else gB
                o = gt[:, (blk % 4) * 256:(blk % 4 + 1) * 256]
                nc.tensor.matmul(o, Whs[:, :, blk * 128:(blk + 1) * 128], hm8,
                                 start=True, stop=False, perf_mode=PM.DoubleRow)
                nc.tensor.matmul(o, W0s[:, :, blk * 128:(blk + 1) * 128], zt,
                                 start=False, stop=True, perf_mode=PM.DoubleRow)
            return gA, gB

        def stats_mms(t):
            h2 = h_ring[t % RING]
            hsq = hsq_tiles[t % 2]
            stp = hdps.tile([128, 512], F32, tag="hd")
            nc.tensor.matmul(stp[:, 0:256], onesmat, h2[:, 0:256], start=True, stop=False)
            nc.tensor.matmul(stp[:, 0:256], onesmat, h2[:, 256:512], start=False, stop=True)
            nc.tensor.matmul(stp[:, 256:512], onesmat, hsq[:, 0:256], start=True, stop=False)
            nc.tensor.matmul(stp[:, 256:512], onesmat, hsq[:, 256:512], start=False, stop=True)
            return stp

        def qrow(row, stp, on_dve):
            qsb = mpool.tile([1, 512], F32, tag="qsb")
            if on_dve:
                nc.vector.tensor_copy(qsb, stp[0:1, 0:512])
            else:
                nc.scalar.activation(qsb, stp[0:1, 0:512], AF.Copy)
            nc.sync.dma_start(out=stats_dram[row:row + 1, :], in_=qsb)

        def newton(u0):
            """rk rows for steps u0..u0+7 from stats_sb; write rk_dram bf16.
            On Pool (slack path, DVE is the bottleneck engine)."""
            st8 = stpool.tile([8, 512], F32, tag="st8")
            nc.sync.dma_start(out=st8, in_=stats_dram[u0:u0 + 8, :])
            rk8 = stpool.tile([8, 512], BF16, tag="rk8")
            mu = stpool.tile([8, 256], F32, tag="mu")
            tmp = stpool.tile([8, 256], F32, tag="tmp")
            v = stpool.tile([8, 256], F32, tag="vv")
            y = stpool.tile([8, 256], F32, tag="yy")
            nc.vector.tensor_scalar(mu, st8[:, 0:256], 1.0 / H, None, OP.mult)
            nc.vector.tensor_scalar(v, st8[:, 256:512], 0.25 / H, LN_EPS, OP.mult, OP.add)
            nc.vector.tensor_tensor(tmp, mu, mu, OP.mult)
            nc.vector.scalar_tensor_tensor(v, tmp, -0.25, v, OP.mult, OP.add)
            yi, vi = y.bitcast(I32), v.bitcast(I32)
            nc.vector.tensor_scalar(yi, vi, 1, None, OP.logical_shift_right)
            nc.vector.tensor_scalar(yi, yi, 0xFFFFFFFF, None, OP.bitwise_xor)
            nc.vector.tensor_scalar(yi, yi, 0x5F3759E0, None, OP.add)
            for it in range(3):
                nc.vector.tensor_tensor(tmp, y, y, OP.mult)
                nc.vector.tensor_tensor(tmp, tmp, v, OP.mult)
                if it < 2:
                    nc.vector.tensor_scalar(tmp, tmp, -0.5, 1.5, OP.mult, OP.add)
                else:   # fold rstd/2 into the last iteration
                    nc.vector.tensor_scalar(tmp, tmp, -0.25, 0.75, OP.mult, OP.add)
                nc.vector.tensor_tensor(y, y, tmp, OP.mult)
            nc.vector.tensor_copy(rk8[:, 0:256], y)
            nc.vector.tensor_tensor(rk8[:, 256:512], mu, y, OP.mult)
            nc.sync.dma_start(out=rk_dram[u0:u0 + 8, :], in_=rk8)

        def rkb_load(u):
            rkb = mpool.tile([128, 512], BF16, tag="rkb")
            nc.sync.dma_start(out=rkb, in_=_bcast_ap(rk_dram[u:u + 1, :]))
            rkb_tiles[u] = rkb

        def zmk(u):
            """z(u) = (h2 - mu2) * rstd2, bf16 [128, 512]; on Pool."""
            rkb = rkb_tiles.pop(u)
            h2 = h_ring[u % RING]
            zu = mpool.tile([128, 512], BF16, tag="zu")
            nc.gpsimd.tensor_tensor(_pair3(zu), _pair3(h2), _row3(rkb[:, 0:256]), OP.mult)
            z = mpool.tile([128, 512], BF16, tag="z")
            nc.gpsimd.tensor_tensor(_pair3(z), _pair3(zu), _row3(rkb[:, 256:512]), OP.subtract)
            z_tiles[u] = z

        def g1_mms(u, z, alt=False):
            if alt:
                y1ps = gps.tile([128, 1024], F32, tag="gA", name="y1alt")
            else:
                y1ps = yps.tile([128, 1024], F32, tag="y1")
            for m in range(4):
                o = y1ps[:, m * 256:(m + 1) * 256]
                nc.tensor.matmul(o, W1s[0][:, m * 128:(m + 1) * 128], z[:, 0:256], start=True, stop=False)
                nc.tensor.matmul(o, W1s[1][:, m * 128:(m + 1) * 128], z[:, 256:512], start=False, stop=False)
                nc.tensor.matmul(o, b1s[0:1, m * 128:(m + 1) * 128], onesrow, start=False, stop=True)
            return y1ps

        def g2_mms(u, e1, alt=False):
            if alt:
                y2ps = gps.tile([128, 512], F32, tag="gB", name="y2alt")
            else:
                y2ps = yps.tile([128, 512], F32, tag="y2")
            for m in range(2):
                o = y2ps[:, m * 256:(m + 1) * 256]
                for k in range(4):
                    nc.tensor.matmul(o, W2s[k][:, m * 128:(m + 1) * 128], e1[:, k * 256:(k + 1) * 256],
                                     start=(k == 0), stop=False)
                nc.tensor.matmul(o, b2s[0:1, m * 128:(m + 1) * 128], onesrow, start=False, stop=True)
            return y2ps

        def heads_mms(u, e2, alt=False):
            if alt:
                hd = gps.tile([128, 512], F32, tag="gA", name="hdalt")
            else:
                hd = hdps.tile([128, 512], F32, tag="hd")
            o = hd[0:128, 0:256]
            nc.tensor.matmul(o, Whds[0][:, :], e2[:, 0:256], start=True, stop=False)
            nc.tensor.matmul(o, Whds[1][:, :], e2[:, 256:512], start=False, stop=False)
            nc.tensor.matmul(o, bhds[0:1, :], onesrow, start=False, stop=True)
            return hd

        def step(t, scan=True):
            # stage indices: w0 zmk, w1 g1/e1x/m1/e1, w3 g2/e2x/m2/e2,
            # w5 heads/osb/ls/mid/scp/out
            w0, w1, w3, w5 = t - LAG + 1, t - LAG, t - LAG - 1, t - LAG - 2
            # ---- PE queue: gates, stats(t-1), g1, heads, g2, [mid later] ----
            if scan:
                with tc.high_priority():
                    gA, gB = gates(t)
            stp = stats_mms(t - 1) if 1 <= t <= T else None
            if scan and t < LAG:
                # fill-phase PE warmup: keep the tensor engine p-state high
                ydum = yps.tile([128, 1024], F32, tag="y1", name="ydum")
                for _d in range(2):
                    nc.tensor.matmul(ydum[:, _d * 512:(_d + 1) * 512],
                                     W1s[_d][:, 0:128], ctile[:, 0:512],
                                     start=True, stop=True)
            y1ps = (g1_mms(w1, z_tiles.pop(w1), alt=(not scan and w1 % 2 == 1))
                    if w1 in z_tiles else None)
            hd = heads_mms(w5, e2_tiles.pop(w5)) if w5 in e2_tiles else None
            y2ps = (g2_mms(w3, e1_tiles.pop(w3), alt=(not scan and w3 % 2 == 1))
                    if w3 in e1_tiles else None)
            # ---- Pool queue: zu, z, [h2, hsq later] ----
            if 0 <= w0 < T:
                zmk(w0)
            if 0 <= t - 10 < T:
                rkb_load(t - 10)
            # ---- ACT queue: tfi, tg, to, osb, tcn, e1x, e2x ----
            if scan:
                with tc.high_priority():
                    tfi = spool.tile([128, 1024], BF16, tag="tfi")
                    nc.scalar.activation(tfi, gA, AF.Tanh, scale=GSC)
                    nc.scalar.activation(ctile[:, 512:1024], gB[:, 0:512], AF.Tanh, scale=GSC)
                    to = spool.tile([128, 512], BF16, tag="to")
                    nc.scalar.activation(to, gB[:, 512:1024], AF.Tanh, scale=GSC)
            if hd is not None:
                osb = mpool.tile([44, 256], F32, tag="osb")
                nc.scalar.activation(osb, hd[0:44, 0:256], AF.Copy)
            # ---- DVE queue: qrow(even), ua, csum, t1, som, ls, hm8 ----
            if stp is not None and t % 2 == 0:
                qrow(t - 1, stp, on_dve=(t % 4 == 0))
            if scan:
                with tc.high_priority():
                    t1 = spool.tile([128, 512], BF16, tag="t1")
                    nc.vector.tensor_scalar(t1, to, 1.0, None, OP.add)

                    som = None
                    if t < T - 1:
                        som = spool.tile([128, 512], BF16, tag="som")
                        nc.vector.tensor_tensor(_pair3(som), _pair3(t1), _row3(state["mb"]), OP.mult)
                    # sig(f) = 0.5*tf + 0.5 ; tip = ti + 1  (both 4x TS on tfi halves)
                    ft = spool.tile([128, 1024], BF16, tag="ft")
                    nc.vector.tensor_scalar(ft[:, 0:512], tfi[:, 0:512], 0.5, 0.5, OP.mult, OP.add)
                    nc.vector.tensor_scalar(ft[:, 512:1024], tfi[:, 512:1024], 1.0, None, OP.add)
                    # ua = (sig(f)|tip) * (c2|tg)  -> a = sig(f)*c2, p = (ti+1)*tg
                    ua = spool.tile([128, 1024], BF16, tag="ua")
                    nc.vector.tensor_tensor(ua, ft, ctile, OP.mult)
                    # c2' = a + p
                    nc.vector.tensor_tensor(ctile[:, 0:512], ua[:, 0:512], ua[:, 512:1024], OP.add)
                    tcn = spool.tile([128, 512], BF16, tag="tc")
                    nc.scalar.activation(tcn, ctile[:, 0:512], AF.Tanh, scale=0.5)
            if hd is not None:
                ls = mpool.tile([12, 256], BF16, tag="ls")
                if scan:
                    nc.vector.tensor_scalar(ls, osb[32:44, :], -5.0, 2.0, OP.max, OP.min)
                else:
                    nc.vector.tensor_scalar(ls, hd[32:44, 0:256], -5.0, 2.0, OP.max, OP.min)
                nc.tensor.matmul(hd[32:33, 256:512], ones12, ls, start=True, stop=True)
            if scan and t < T - 1:
                with tc.high_priority():
                    hm8n = spool.tile([128, 2, 256], F8, tag="hm8")
                    nc.vector.tensor_tensor(hm8n, _pair3(som), _pair3(tcn), OP.mult)
                state["hm8"] = hm8n
            if scan:
                h2 = h_ring[t % RING]
                nc.vector.tensor_tensor(h2, t1, tcn, OP.mult)
            if stp is not None and t % 2 == 1:
                qrow(t - 1, stp, on_dve=(t % 4 == 0))
            if hd is not None:
                scp = mpool.tile([1, 256], F32, tag="scp")
                nc.scalar.activation(scp, hd[32:33, 256:512], AF.Copy)
                nc.sync.dma_start(out=out_d[0:12, w5 * BL:(w5 + 1) * BL], in_=osb[0:12, :])
                nc.sync.dma_start(out=s_dram[w5:w5 + 1, :], in_=scp)

            # ---- MLP elementwise tails (long slack) ----
            if y1ps is not None:
                e1x = mpool.tile([128, 1024], BF16, tag="e1x")
                nc.scalar.activation(e1x[:, 0:512], y1ps[:, 0:512], AF.Exp)
                nc.scalar.activation(e1x[:, 512:1024], y1ps[:, 512:1024], AF.Exp)
                m1 = mpool.tile([128, 1024], BF16, tag="m1")
                nc.vector.tensor_scalar(m1, e1x, 1.0, None, OP.min)
                e1 = mpool.tile([128, 1024], BF16, tag="e1")
                nc.vector.scalar_tensor_tensor(e1[:, 0:512], y1ps[:, 0:512], 1.0,
                                               m1[:, 0:512], OP.add, OP.max)
                nc.vector.scalar_tensor_tensor(e1[:, 512:1024], y1ps[:, 512:1024], 1.0,
                                               m1[:, 512:1024], OP.add, OP.max)
                e1_tiles[w1] = e1
            if y2ps is not None:
                e2x = mpool.tile([128, 512], BF16, tag="e2x")
                nc.scalar.activation(e2x, y2ps, AF.Exp)
                m2 = mpool.tile([128, 512], BF16, tag="m2")
                nc.vector.tensor_scalar(m2, e2x, 1.0, None, OP.min)
                e2 = mpool.tile([128, 512], BF16, tag="e2")
                nc.vector.scalar_tensor_tensor(e2, y2ps, 1.0, m2, OP.add, OP.max)
                e2_tiles[w3] = e2
            if t == 9 or (15 <= t <= 63 and (t - 15) % 8 == 0) or t == 65:
                with tc.high_priority(offset=300):
                    newton(min(t - 9, T - 8))
            if scan:
                hsq = hsq_tiles[t % 2]
                nc.gpsimd.tensor_tensor(hsq, h2, h2, OP.mult)
                if t % ZCH == ZCH // 2 and t + ZCH // 2 < T:
                    kchunk = (t + ZCH // 2) // ZCH
                    zcn = zpool.tile([32, ZCH * 512], F8, tag="zc", name="zcn")
                    nc.sync.dma_start(out=zcn,
                                      in_=z0_d[:, kchunk * ZCH * 512:(kchunk + 1) * ZCH * 512])
                    state["zc_next"] = zcn
                if t < T - 1:
                    mbn = spool.tile([128, BL], BF16, tag="mb")
                    nc.sync.dma_start(out=mbn, in_=_bcast_ap(mbrow_d[t + 1:t + 2, :]))
                    state["mb"] = mbn
                if t % ZCH == ZCH - 1 and t < T - 1:
                    state["zc"] = state["zc_next"]

        def elu1_drain(w, y1ps):
            e1x = mpool.tile([128, 1024], BF16, tag="e1x")
            nc.scalar.activation(e1x[:, 0:512], y1ps[:, 0:512], AF.Exp)
            nc.scalar.activation(e1x[:, 512:1024], y1ps[:, 512:1024], AF.Exp)
            m1 = mpool.tile([128, 1024], BF16, tag="m1")
            nc.vector.tensor_scalar(m1, e1x, 1.0, None, OP.min)
            e1 = mpool.tile([128, 1024], BF16, tag="e1")
            nc.vector.scalar_tensor_tensor(e1[:, 0:512], y1ps[:, 0:512], 1.0,
                                           m1[:, 0:512], OP.add, OP.max)
            nc.vector.scalar_tensor_tensor(e1[:, 512:1024], y1ps[:, 512:1024], 1.0,
                                           m1[:, 512:1024], OP.add, OP.max)
            return e1

        def elu2_drain(w, y2ps):
            e2x = mpool.tile([128, 512], BF16, tag="e2x")
            nc.scalar.activation(e2x, y2ps, AF.Exp)
            m2 = mpool.tile([128, 512], BF16, tag="m2")
            nc.vector.tensor_scalar(m2, e2x, 1.0, None, OP.min)
            e2 = mpool.tile([128, 512], BF16, tag="e2")
            nc.vector.scalar_tensor_tensor(e2, y2ps, 1.0, m2, OP.add, OP.max)
            return e2

        def finish_heads(w, e2t):
            hd = heads_mms(w, e2t)
            osb = mpool.tile([44, 256], F32, tag="osb")
            nc.scalar.activation(osb, hd[0:44, 0:256], AF.Copy)
            ls = mpool.tile([12, 256], BF16, tag="ls")
            nc.vector.tensor_scalar(ls, hd[32:44, 0:256], -5.0, 2.0, OP.max, OP.min)
            nc.tensor.matmul(hd[32:33, 256:512], ones12, ls, start=True, stop=True)
            scp = mpool.tile([1, 256], F32, tag="scp")
            nc.scalar.activation(scp, hd[32:33, 256:512], AF.Copy)
            nc.sync.dma_start(out=out_d[0:12, w * BL:(w + 1) * BL], in_=osb[0:12, :])
            nc.sync.dma_start(out=s_dram[w:w + 1, :], in_=scp)

        for t in range(T):
            step(t)
        for t in range(T, T + LAG + 3):
            step(t, scan=False)
        # logp / ent rows
        s_all = singles.tile([64, 256], F32)
        nc.sync.dma_start(out=s_all, in_=s_dram[:, :])
        lp = singles.tile([64, 256], F32)
        nc.vector.tensor_scalar(lp, s_all, -1.0, C_LOGP, OP.mult, OP.add)
        en = singles.tile([64, 256], F32)
        nc.vector.tensor_scalar(en, s_all, C_ENT, None, OP.add)
        nc.sync.dma_start(out=out_d[12:13, :].rearrange("o (t b) -> (o t) b", t=64), in_=lp)
        nc.sync.dma_start(out=out_d[13:14, :].rearrange("o (t b) -> (o t) b", t=64), in_=en)
    nc.finalize()
    return nc


_NC_CACHE = None


def kernel(x, h0, c0, W_ih, W_hh, b_ih, b_hh, ln_g, ln_b,
           W1, b1, W2, b2, Wm, bm, Ws, bs, done):
    global _NC_CACHE
    F8NP = ml_dtypes.float8_e4m3
    BFNP = ml_dtypes.bfloat16
    x = np.asarray(x, np.float32)
    done_f = np.asarray(done, np.float32)
    keep = 1.0 - done_f

    # gate weights, psum-block order [f0 f1 i0 i1 g0 g1 o0 o1], x16 total scale
    W_hh_f = np.asarray(W_hh, np.float32)
    W_ih_f = np.asarray(W_ih, np.float32)
    bsum = np.asarray(b_ih, np.float32) + np.asarray(b_hh, np.float32)
    gate_scale = np.ones(8, np.float32) * SC
    gate_scale[4:6] *= 2.0          # g blocks predoubled
    # Whp[p, k, blk*128+q] = scale * W_hh[PYCHUNK[blk]*128+q, k*128+p]
    Whp = np.zeros((128, 2, 1024), np.float32)
    W0f = np.zeros((64, 1024), np.float32)    # rows: z-vec, cols: blk*128+q
    for blk in range(8):
        rows = slice(PYCHUNK[blk] * 128, PYCHUNK[blk] * 128 + 128)
        wblk = W_hh_f[rows, :] * gate_scale[blk]          # [128q, 256h]
        Whp[:, 0, blk * 128:(blk + 1) * 128] = wblk[:, 0:128].T
        Whp[:, 1, blk * 128:(blk + 1) * 128] = wblk[:, 128:256].T
        W0f[0:OBS, blk * 128:(blk + 1) * 128] = W_ih_f[rows, :].T * gate_scale[blk]
        W0f[49, blk * 128:(blk + 1) * 128] = bsum[rows] * gate_scale[blk]
    # done mask on f blocks (blk 0,1 = W0f cols 0:256): product -BIG*SC^2*done
    W0f[48, 0:256] = -BIG * SC
    W0dr = np.zeros((32, 2, 1024), np.float32)
    W0dr[:, 0, :] = W0f[0:32, :]
    W0dr[:, 1, :] = W0f[32:64, :]

    # z0: [32, t, 2, 256]; z rows r = k*32+p: 0:48 = SC*x, 48 = SC*done, 49 = SC
    zvec = np.zeros((64, T, BL * NC_N), np.float32)
    zvec[0:OBS] = SC * x.transpose(2, 0, 1)
    zvec[48] = SC * done_f
    zvec[49] = SC

    # ln folded into W1
    W1f = (np.asarray(ln_g, np.float32)[:, None] * np.asarray(W1, np.float32))
    b1f = np.asarray(b1, np.float32) + np.asarray(ln_b, np.float32) @ np.asarray(W1, np.float32)
    W2f = np.asarray(W2, np.float32)
    b2f = np.asarray(b2, np.float32) - W2f.sum(axis=0)
    Whd = np.zeros((M2, 128), np.float32)
    Whd[:, 0:12] = np.asarray(Wm, np.float32)
    Whd[:, 32:44] = np.asarray(Ws, np.float32)
    bhd = np.zeros((1, 128), np.float32)
    bhd[0, 0:12] = np.asarray(bm, np.float32) - np.asarray(Wm, np.float32).sum(axis=0)
    bhd[0, 32:44] = np.asarray(bs, np.float32) - np.asarray(Ws, np.float32).sum(axis=0)

    shared = dict(
        W0dr=W0dr.reshape(32, 2048).astype(F8NP),
        Whdr=Whp.reshape(128, 2048).astype(F8NP),
        W1=W1f.astype(BFNP), W2=W2f.astype(BFNP), Whd=Whd.astype(BFNP),
        b1r=b1f[None, :], b2r=b2f[None, :], bhdr=bhd,
        onesrow=np.ones((1, BL), np.float32),
        onesmat=np.ones((128, 128), np.float32).astype(BFNP),
    )

    def pair(mat):  # [BL, H] -> [128, 512] pair layout of mat.T
        mT = mat.T.astype(np.float32)            # [H, BL]
        return mT.reshape(2, 128, BL).transpose(1, 0, 2).reshape(128, 2 * BL)

    in_maps = []
    for c in range(NC_N):
        sl = slice(c * BL, (c + 1) * BL)
        zc = zvec[:, :, sl]                                  # [64, T, 256]
        z0 = zc.reshape(2, 32, T, BL).transpose(1, 2, 0, 3)  # [32, T, 2, 256]
        mbrow = np.zeros((T, BL), np.float32)
        mbrow[0:T - 1] = 2.0 * keep[1:T, sl]
        hm0 = pair(np.asarray(h0, np.float32)[sl] * (SC * keep[0, sl])[:, None])
        c0p = pair(2.0 * np.asarray(c0, np.float32)[sl])
        m = dict(z0=z0.reshape(32, T * 512).astype(F8NP),
                 mbrow=mbrow.astype(BFNP),
                 hm0=hm0.astype(F8NP), c0p=c0p.astype(BFNP), **shared)
        in_maps.append(m)

    if _NC_CACHE is None:
        _NC_CACHE = build_nc()
    res = run_bass_kernel_spmd(_NC_CACHE, in_maps, core_ids=list(range(NC_N)))
    full = np.empty((T, B, 14), np.float32)
    for c in range(NC_N):
        oc = res.results[c]["out"].reshape(14, T, BL)
        full[:, c * BL:(c + 1) * BL, :] = oc.transpose(1, 2, 0)
    return full.reshape(T * B, 14)
